# revision 16
# baseline (speedup 1.0000x reference)
"""Trainium2 Bass kernel for a pre-norm transformer block (attention + MLP).

Problem: x [2, 4096, 768] fp32 through
    x = x + proj(attn(LN1(x)))
    x = x + W2 @ gelu(W1 @ LN2(x))
on 8 NeuronCores.

Sharding: core c handles batch b = c // 4 and sequence slice g = c % 4
(1024 tokens). Each core computes QKV for its own tokens, K/V are
all-gathered within each 4-core batch group (one AllGather through DRAM
bounce buffers), attention/proj/LN2/MLP are fully sequence-parallel.

Layout: activations are kept feature-major ([feature, token], features on
SBUF partitions) so every linear layer is a chain of 128x512 matmuls with
no transposes. LayerNorm statistics (sums over the feature = partition dim)
are computed with ones-vector matmuls on the tensor engine; per-token
scale/shift vectors are broadcast across partitions with GPSIMD
partition_broadcast. Softmax runs on transposed scores ([key, query]) so
that exp'd probabilities feed the AV matmul directly as the moving operand;
the softmax denominator falls out of the AV matmul itself via a ones column
appended to V. LN affine params (g, beta) and the 1/sqrt(hd) query scale
are folded into the weight matrices on the host.
"""

import numpy as np
import ml_dtypes

import concourse.bass as bass
import concourse.tile as tile
from concourse import bacc, mybir
from concourse import bass_utils

F32 = mybir.dt.float32
BF16 = mybir.dt.bfloat16
NPBF16 = ml_dtypes.bfloat16
AF = mybir.ActivationFunctionType

D = 768
NH = 12
HD = 64
NMLP = 3072
B = 2
T = 4096
EPS = 1e-6
NCORES = 8
GROUPS = [[0, 1, 2, 3], [4, 5, 6, 7]]

TC = T // 4            # tokens per core (1024)
NCH = D // 128         # 6 feature chunks
NPAIR = NH // 2        # 6 head pairs
QKV_CH = 3 * D // 128  # 18
MLP_CH = NMLP // 128   # 24
NTK = T // 128         # 32 key tiles (full sequence)
NTJ = TC // 128        # 8 own-token tiles
HALF = 512             # matmul free-dim tile (one PSUM bank of fp32)
NHALF = TC // HALF     # 2

_CACHE: dict = {}


# --------------------------------------------------------------------------
# device program
# --------------------------------------------------------------------------

def _ln_feature_major(tc, nc, pools, x_tiles, h_tiles_out):
    """LayerNorm over the feature (partition) dim of 6 x [128, TC] fp32 tiles.

    Writes normalized bf16 into h_tiles_out (6 tiles [128, TC]).
    Affine (g, beta) is folded into the following matmul's weights on host.
    """
    sq_pool, norm_pool, stat_psum, small = pools
    ones_f32 = small["ones_f32"]
    ones_bf16 = small["ones_bf16"]

    sq_tiles = []
    for c in range(NCH):
        sq = sq_pool.tile([128, TC], BF16, tag="sq")
        nc.vector.tensor_mul(sq, x_tiles[c], x_tiles[c])
        sq_tiles.append(sq)

    ps_x = [stat_psum.tile([1, HALF], F32, tag="st_x", name="ps_x") for _ in range(NHALF)]
    ps_q = [stat_psum.tile([1, HALF], F32, tag="st_q", name="ps_q") for _ in range(NHALF)]
    for h in range(NHALF):
        sl = slice(h * HALF, (h + 1) * HALF)
        for c in range(NCH):
            nc.tensor.matmul(ps_x[h], ones_f32, x_tiles[c][:, sl],
                             start=(c == 0), stop=(c == NCH - 1))
        for c in range(NCH):
            nc.tensor.matmul(ps_q[h], ones_bf16, sq_tiles[c][:, sl],
                             start=(c == 0), stop=(c == NCH - 1))

    mu = norm_pool.tile([1, TC], F32, tag="mu")
    m2 = norm_pool.tile([1, TC], F32, tag="m2")
    var = norm_pool.tile([1, TC], F32, tag="var")
    lnv = norm_pool.tile([1, TC], F32, tag="lnv")
    rs = norm_pool.tile([1, TC], F32, tag="rs")
    nb = norm_pool.tile([1, TC], F32, tag="nb")
    for h in range(NHALF):
        sl = slice(h * HALF, (h + 1) * HALF)
        nc.vector.tensor_scalar_mul(mu[:, sl], ps_x[h], 1.0 / D)
        nc.vector.tensor_scalar_mul(m2[:, sl], ps_q[h], 1.0 / D)
    nc.vector.tensor_mul(var, mu, mu)
    nc.vector.tensor_sub(var, m2, var)
    # rs = (var + eps) ** -0.5 via Ln/Exp (both live in one ACT table set)
    nc.scalar.activation(lnv, var, AF.Ln, bias=small["eps"])
    nc.scalar.activation(rs, lnv, AF.Exp, scale=-0.5)
    nc.vector.tensor_mul(nb, mu, rs)
    nc.vector.tensor_scalar_mul(nb, nb, -1.0)

    a_bc = norm_pool.tile([128, TC], F32, tag="a_bc")
    b_bc = norm_pool.tile([128, TC], F32, tag="b_bc")
    nc.gpsimd.partition_broadcast(a_bc, rs)
    nc.gpsimd.partition_broadcast(b_bc, nb)

    for c in range(NCH):
        t = norm_pool.tile([128, TC], F32, tag="norm_tmp")
        nc.vector.tensor_mul(t, x_tiles[c], a_bc)
        nc.vector.tensor_add(h_tiles_out[c], t, b_bc)


def _emit(tc, nc, io):
    xT, wqkvT, wprojT, w1T, w2T, bqkv, b1, outT = (
        io["xT"], io["wqkvT"], io["wprojT"], io["w1T"], io["w2T"],
        io["bqkv"], io["b1"], io["outT"])

    xT_v = xT.rearrange("(c p) t -> c p t", p=128)
    out_v = outT.rearrange("(c p) t -> c p t", p=128)

    const = tc.alloc_tile_pool(name="const", bufs=1)
    dram = tc.alloc_tile_pool(name="dram", bufs=1, space="DRAM")

    ident = const.tile([128, 128], F32)
    from concourse.masks import make_identity
    make_identity(nc, ident)
    ones_f32 = const.tile([128, 1], F32)
    ones_bf16 = const.tile([128, 1], BF16)
    nc.any.memset(ones_f32, 1.0)
    nc.any.memset(ones_bf16, 1.0)
    bqkv_sb = const.tile([128, QKV_CH], F32)
    b1_sb = const.tile([128, MLP_CH], F32)
    nc.sync.dma_start(bqkv_sb, bqkv[:])
    nc.sync.dma_start(b1_sb, b1[:])
    wproj_sb = const.tile([128, NCH, D], BF16)
    nc.sync.dma_start(wproj_sb, wprojT.rearrange("(c p) o -> p c o", p=128))
    eps_sb = const.tile([1, 1], F32)
    nc.any.memset(eps_sb, EPS)
    small = {"ones_f32": ones_f32, "ones_bf16": ones_bf16, "eps": eps_sb}

    # DRAM bounce buffers for the K/V all-gather (within 4-core batch group)
    kv_k_in = dram.tile([D, TC], BF16)
    kv_k_out = dram.tile([4 * D, TC], BF16)
    kv_v_in = dram.tile([TC, NH * (HD + 1)], BF16)
    kv_v_out = dram.tile([T, NH * (HD + 1)], BF16)

    # persistent activation pools (alloc order = reverse release order)
    hp = tc.alloc_tile_pool(name="hp", bufs=NCH)
    x2_pool = tc.alloc_tile_pool(name="x2", bufs=NCH)
    xin = tc.alloc_tile_pool(name="xin", bufs=1)
    attno = tc.alloc_tile_pool(name="attno", bufs=NPAIR)
    qt = tc.alloc_tile_pool(name="qt", bufs=NPAIR)

    xin_t = xin.tile([128, NCH, TC], F32, tag="xin")
    nc.sync.dma_start(xin_t, xT.rearrange("(c p) t -> p c t", p=128))
    x_tiles = [xin_t[:, c, :] for c in range(NCH)]

    # ---------------- Phase A: LN1 ----------------
    h_tiles = [hp.tile([128, TC], BF16, tag="h", name="h1t") for _ in range(NCH)]
    with (
        tc.tile_pool(name="sq1", bufs=2) as sq_pool,
        tc.tile_pool(name="norm1", bufs=2) as norm_pool,
        tc.tile_pool(name="stat1", bufs=2, space="PSUM") as stat_psum,
    ):
        _ln_feature_major(tc, nc, (sq_pool, norm_pool, stat_psum, small),
                          x_tiles, h_tiles)

    # ---------------- Phase B: QKV + v transpose + bounce out ----------------
    kv_k_in_v = kv_k_in.rearrange("(c p) t -> c p t", p=128)
    kv_v_in_v = kv_v_in.rearrange("(j p) f -> j p f", p=128)
    with (
        tc.tile_pool(name="wqkv", bufs=1) as wq_pool,
        tc.tile_pool(name="kvloc", bufs=2) as kv_loc,
        tc.tile_pool(name="vtok", bufs=NTJ) as vtok_pool,
        tc.tile_pool(name="mmB", bufs=4, space="PSUM") as mm_psum,
        tc.tile_pool(name="tpB", bufs=2, space="PSUM") as tp_psum,
    ):
        wq_sb = wq_pool.tile([128, NCH, 3 * D], BF16)
        nc.sync.dma_start(wq_sb, wqkvT.rearrange("(c p) o -> p c o", p=128))

        v_tok = []
        for j in range(NTJ):
            vt = vtok_pool.tile([128, NH, HD + 1], BF16, tag="vtok")
            nc.any.memset(vt[:, :, HD:HD + 1], 1.0)
            v_tok.append(vt)

        q_tiles = []
        for oc in range(QKV_CH):
            if oc < NPAIR:
                dst = qt.tile([128, TC], BF16, tag="qt")
                q_tiles.append(dst)
            elif oc < 2 * NPAIR:
                dst = kv_loc.tile([128, TC], BF16, tag="kloc")
            else:
                dst = kv_loc.tile([128, TC], F32, tag="vloc")
            for h in range(NHALF):
                sl = slice(h * HALF, (h + 1) * HALF)
                ps = mm_psum.tile([128, HALF], F32, tag="mm")
                for c in range(NCH):
                    nc.tensor.matmul(ps, wq_sb[:, c, oc * 128:(oc + 1) * 128],
                                     h_tiles[c][:, sl],
                                     start=(c == 0), stop=(c == NCH - 1))
                nc.vector.tensor_scalar_add(dst[:, sl], ps,
                                            bqkv_sb[:, oc:oc + 1])
            if NPAIR <= oc < 2 * NPAIR:
                nc.sync.dma_start(kv_k_in_v[oc - NPAIR], dst)
            elif oc >= 2 * NPAIR:
                c = oc - 2 * NPAIR  # v feature chunk (heads 2c, 2c+1)
                for j in range(NTJ):
                    tp = tp_psum.tile([128, 128], F32, tag="tp")
                    nc.tensor.transpose(tp, dst[:, j * 128:(j + 1) * 128],
                                        ident)
                    nc.vector.tensor_copy(
                        v_tok[j][:, 2 * c:2 * c + 2, 0:HD],
                        tp.rearrange("p (a b) -> p a b", a=2))
        for j in range(NTJ):
            nc.sync.dma_start(kv_v_in_v[j], v_tok[j])

    # ---------------- Phase C: all-gather K/V ----------------
    nc.gpsimd.collective_compute(
        "AllGather", mybir.AluOpType.bypass, replica_groups=GROUPS,
        ins=[kv_k_in[:]], outs=[kv_k_out[:]])
    nc.gpsimd.collective_compute(
        "AllGather", mybir.AluOpType.bypass, replica_groups=GROUPS,
        ins=[kv_v_in[:]], outs=[kv_v_out[:]])

    kv_k_out_v = kv_k_out.rearrange("(g c p) t -> g c p t", c=NCH, p=128)
    kv_v_out_v = kv_v_out.rearrange("(g j p) f -> g j p f", j=NTJ, p=128)

    # ---------------- Phase D: attention ----------------
    with (
        tc.tile_pool(name="vfull", bufs=1) as vfull_pool,
        tc.tile_pool(name="ktp", bufs=2) as ktp_pool,
        tc.tile_pool(name="attn", bufs=3) as attn_pool,
        tc.tile_pool(name="attn_eps", bufs=2) as eps_pool,
        tc.tile_pool(name="qk", bufs=2, space="PSUM") as qk_psum,
        tc.tile_pool(name="av", bufs=4, space="PSUM") as av_psum,
    ):
        vf_t = vfull_pool.tile([128, NTK, NH * (HD + 1)], BF16, tag="vfull")
        vsrc = kv_v_out.rearrange("(g j p) f -> g p j f", j=NTJ, p=128)
        for g in range(4):
            nc.sync.dma_start(vf_t[:, g * NTJ:(g + 1) * NTJ, :], vsrc[g])
        vf_v = vf_t.rearrange("p t (h f) -> p t h f", h=NH)
        v_full = [vf_v[:, t] for t in range(NTK)]

        attno_tiles = []
        for p in range(NPAIR):
            ktp = ktp_pool.tile([128, T], BF16, tag="ktp")
            ksrc = kv_k_out.rearrange("(g c p) t -> p c g t", c=NCH, p=128)
            nc.sync.dma_start(ktp.rearrange("p (g t) -> p g t", g=4),
                              ksrc[:, p])

            avs = [av_psum.tile([HD + 1, HALF], F32, tag="av", name="avt")
                   for _ in range(4)]  # A0 A1 B0 B1
            for ch in range(NTK):
                ksl = slice(ch * 128, (ch + 1) * 128)
                for hh in range(2):
                    head = 2 * p + hh
                    lhs = ktp[hh * 64:(hh + 1) * 64, ksl]
                    ps = qk_psum.tile([128, TC], F32, tag="qk")
                    for h in range(NHALF):
                        nc.tensor.matmul(
                            ps[:, h * HALF:(h + 1) * HALF],
                            lhs,
                            q_tiles[p][hh * 64:(hh + 1) * 64,
                                       h * HALF:(h + 1) * HALF],
                            start=True, stop=True)  # noqa: E501
                    at = attn_pool.tile([128, TC], BF16, tag="attn")
                    nc.scalar.activation(at, ps, AF.Exp)
                    for h in range(NHALF):
                        nc.tensor.matmul(
                            avs[2 * hh + h],
                            v_full[ch][:, head, :],
                            at[:, h * HALF:(h + 1) * HALF],
                            start=(ch == 0), stop=(ch == NTK - 1),
                            skip_group_check=True)

            ao = attno.tile([128, TC], BF16, tag="attno")
            for hh in range(2):
                # drain AV psum to SBUF (DMA cannot read PSUM and DVE cannot
                # shift partitions, so stage through an aligned SBUF copy)
                av_sb = eps_pool.tile([HD + 1, TC], F32, tag="av_sb")
                for h in range(NHALF):
                    sl = slice(h * HALF, (h + 1) * HALF)
                    nc.vector.tensor_copy(av_sb[:, sl], avs[2 * hh + h])
                rd_raw = eps_pool.tile([1, TC], F32, tag="rd_raw")
                rd = eps_pool.tile([1, TC], F32, tag="rden")
                nc.sync.dma_start(rd_raw, av_sb[HD:HD + 1, :])
                nc.vector.reciprocal(rd, rd_raw)
                bc = eps_pool.tile([HD, TC], F32, tag="bc")
                nc.gpsimd.partition_broadcast(bc, rd)
                if hh == 0:
                    nc.vector.tensor_mul(ao[0:HD, :], av_sb[0:HD, :], bc)
                else:
                    tmp = eps_pool.tile([HD, TC], BF16, tag="tmpB")
                    nc.vector.tensor_mul(tmp, av_sb[0:HD, :], bc)
                    nc.sync.dma_start(ao[HD:128, :], tmp)
            attno_tiles.append(ao)

    qt.release()

    # ---------------- Phase E: proj + residual + LN2 ----------------
    x2_tiles = []
    with tc.tile_pool(name="prE", bufs=4, space="PSUM") as pr_psum:
        for oc in range(NCH):
            x2t = x2_pool.tile([128, TC], F32, tag="x2")
            for h in range(NHALF):
                sl = slice(h * HALF, (h + 1) * HALF)
                ps = pr_psum.tile([128, HALF], F32, tag="pr")
                for p in range(NPAIR):
                    nc.tensor.matmul(ps,
                                     wproj_sb[:, p, oc * 128:(oc + 1) * 128],
                                     attno_tiles[p][:, sl],
                                     start=(p == 0), stop=(p == NPAIR - 1))
                nc.vector.tensor_add(x2t[:, sl], x_tiles[oc][:, sl], ps)
            x2_tiles.append(x2t)

    attno.release()
    xin.release()
    w2_pool = tc.alloc_tile_pool(name="w2", bufs=1)
    w2_sb = w2_pool.tile([128, MLP_CH, D], BF16)
    nc.sync.dma_start(w2_sb, w2T.rearrange("(c p) o -> p c o", p=128))



    h2_tiles = [hp.tile([128, TC], BF16, tag="h", name="h2t") for _ in range(NCH)]
    with (
        tc.tile_pool(name="sq2", bufs=2) as sq_pool,
        tc.tile_pool(name="norm2", bufs=2) as norm_pool,
        tc.tile_pool(name="stat2", bufs=2, space="PSUM") as stat_psum,
    ):
        _ln_feature_major(tc, nc, (sq_pool, norm_pool, stat_psum, small),
                          x2_tiles, h2_tiles)

    # ---------------- Phase F: MLP ----------------
    with (
        tc.tile_pool(name="mid", bufs=MLP_CH) as mid_pool,
        tc.tile_pool(name="outp", bufs=2) as out_pool,
        tc.tile_pool(name="mmF", bufs=4, space="PSUM") as mm_psum,
    ):
        with tc.tile_pool(name="w1", bufs=1) as w1_pool:
            w1_sb = w1_pool.tile([128, NCH, NMLP], BF16)
            nc.sync.dma_start(w1_sb, w1T.rearrange("(c p) o -> p c o", p=128))

            mid_tiles = []
            for oc in range(MLP_CH):
                mt = mid_pool.tile([128, TC], BF16, tag="mid")
                for h in range(NHALF):
                    sl = slice(h * HALF, (h + 1) * HALF)
                    ps = mm_psum.tile([128, HALF], F32, tag="mm")
                    for c in range(NCH):
                        nc.tensor.matmul(
                            ps, w1_sb[:, c, oc * 128:(oc + 1) * 128],
                            h2_tiles[c][:, sl],
                            start=(c == 0), stop=(c == NCH - 1))
                    nc.scalar.activation(mt[:, sl], ps, AF.Gelu,
                                         bias=b1_sb[:, oc:oc + 1])
                mid_tiles.append(mt)

        for oc in range(NCH):
            ot = out_pool.tile([128, TC], F32, tag="out")
            for h in range(NHALF):
                sl = slice(h * HALF, (h + 1) * HALF)
                ps = mm_psum.tile([128, HALF], F32, tag="mm")
                for c in range(MLP_CH):
                    nc.tensor.matmul(ps, w2_sb[:, c, oc * 128:(oc + 1) * 128],
                                     mid_tiles[c][:, sl],
                                     start=(c == 0), stop=(c == MLP_CH - 1))
                nc.vector.tensor_add(ot[:, sl], x2_tiles[oc][:, sl], ps)
            nc.sync.dma_start(out_v[oc], ot)

    for pool in (w2_pool, x2_pool, hp, dram, const):
        pool.release()


def _build():
    nc = bacc.Bacc("TRN2", target_bir_lowering=False, debug=False,
                   num_devices=NCORES)
    io = {
        "xT": nc.dram_tensor("xT", [D, TC], F32, kind="ExternalInput").ap(),
        "wqkvT": nc.dram_tensor("wqkvT", [D, 3 * D], BF16,
                                kind="ExternalInput").ap(),
        "wprojT": nc.dram_tensor("wprojT", [D, D], BF16,
                                 kind="ExternalInput").ap(),
        "w1T": nc.dram_tensor("w1T", [D, NMLP], BF16,
                              kind="ExternalInput").ap(),
        "w2T": nc.dram_tensor("w2T", [NMLP, D], BF16,
                              kind="ExternalInput").ap(),
        "bqkv": nc.dram_tensor("bqkv", [128, QKV_CH], F32,
                               kind="ExternalInput").ap(),
        "b1": nc.dram_tensor("b1", [128, MLP_CH], F32,
                             kind="ExternalInput").ap(),
        "outT": nc.dram_tensor("outT", [D, TC], F32,
                               kind="ExternalOutput").ap(),
    }
    with tile.TileContext(nc) as tc:
        _emit(tc, nc, io)
    nc.compile()
    return nc


def _get_nc():
    if "nc" not in _CACHE:
        _CACHE["nc"] = _build()
    return _CACHE["nc"]


# --------------------------------------------------------------------------
# host side
# --------------------------------------------------------------------------

def _prep_in_maps(x, W_qkv, b_qkv, W_proj, b_proj, W1, b1, W2, b2,
                  g1, beta1, g2, beta2):
    f32 = np.float32
    x = np.asarray(x, f32)
    W_qkv = np.asarray(W_qkv, f32)
    b_qkv = np.asarray(b_qkv, f32)
    W_proj = np.asarray(W_proj, f32)
    b_proj = np.asarray(b_proj, f32)
    W1 = np.asarray(W1, f32)
    b1 = np.asarray(b1, f32)
    W2 = np.asarray(W2, f32)
    b2 = np.asarray(b2, f32)
    g1 = np.asarray(g1, f32)
    beta1 = np.asarray(beta1, f32)
    g2 = np.asarray(g2, f32)
    beta2 = np.asarray(beta2, f32)

    assert np.all(b_proj == 0) and np.all(b2 == 0), \
        "nonzero proj/fc2 bias not supported by this kernel build"

    scale = HD ** -0.5
    Wq = W_qkv * g1[None, :]
    bq = b_qkv + W_qkv @ beta1
    Wq[:D] *= scale
    bq = bq.copy()
    bq[:D] *= scale

    W1e = W1 * g2[None, :]
    b1e = b1 + W1 @ beta2

    wqkvT = np.ascontiguousarray(Wq.T).astype(NPBF16)
    wprojT = np.ascontiguousarray(W_proj.T).astype(NPBF16)
    w1T = np.ascontiguousarray(W1e.T).astype(NPBF16)
    w2T = np.ascontiguousarray(W2.T).astype(NPBF16)
    bqkv_dev = np.ascontiguousarray(bq.reshape(QKV_CH, 128).T).astype(f32)
    b1_dev = np.ascontiguousarray(b1e.reshape(MLP_CH, 128).T).astype(f32)

    shared = {"wqkvT": wqkvT, "wprojT": wprojT, "w1T": w1T, "w2T": w2T,
              "bqkv": bqkv_dev, "b1": b1_dev}
    in_maps = []
    for c in range(NCORES):
        b, g = divmod(c, 4)
        xT = np.ascontiguousarray(x[b, g * TC:(g + 1) * TC, :].T)
        in_maps.append({"xT": xT, **shared})
    return in_maps


def _assemble(results):
    out = np.empty((B, T, D), np.float32)
    for c in range(NCORES):
        b, g = divmod(c, 4)
        out[b, g * TC:(g + 1) * TC, :] = results[c]["outT"].T
    return out


def kernel(**inputs) -> np.ndarray:
    in_maps = _prep_in_maps(**inputs)
    nc = _get_nc()
    res = bass_utils.run_bass_kernel_spmd(
        nc, in_maps, core_ids=list(range(NCORES)))
    _CACHE["last_results"] = res
    return _assemble(res.results)


def kernel_sim(**inputs) -> np.ndarray:
    """Run through MultiCoreSim instead of hardware (for testing/timing)."""
    from concourse.bass_interp import MultiCoreSim
    in_maps = _prep_in_maps(**inputs)
    nc = _get_nc()
    sim = MultiCoreSim(nc, num_cores=NCORES, trace=False)
    for c in range(NCORES):
        for name, arr in in_maps[c].items():
            sim.cores[c].tensor(name)[:] = arr
    sim.simulate(check_with_hw=False)
    _CACHE["sim_time_ns"] = sim.global_time
    results = [{"outT": sim.cores[c].mem_tensor("outT")} for c in range(NCORES)]
    return _assemble(results)


# revision 17
# speedup vs baseline: 4229.8201x; 4229.8201x over previous
"""Trainium2 Bass kernel for a pre-norm transformer block (attention + MLP).

Problem: x [2, 4096, 768] fp32 through
    x = x + proj(attn(LN1(x)))
    x = x + W2 @ gelu(W1 @ LN2(x))
on 8 NeuronCores.

Sharding: core c handles batch b = c // 4 and sequence slice g = c % 4
(1024 tokens). Each core computes QKV for its own tokens, K/V are
all-gathered within each 4-core batch group (one AllGather through DRAM
bounce buffers), attention/proj/LN2/MLP are fully sequence-parallel.

Layout: activations are kept feature-major ([feature, token], features on
SBUF partitions) so every linear layer is a chain of 128x512 matmuls with
no transposes. LayerNorm statistics (sums over the feature = partition dim)
are computed with ones-vector matmuls on the tensor engine; per-token
scale/shift vectors are broadcast across partitions with GPSIMD
partition_broadcast. Softmax runs on transposed scores ([key, query]) so
that exp'd probabilities feed the AV matmul directly as the moving operand;
the softmax denominator falls out of the AV matmul itself via a ones column
appended to V. LN affine params (g, beta) and the 1/sqrt(hd) query scale
are folded into the weight matrices on the host.
"""

import numpy as np
import ml_dtypes

import concourse.bass as bass
import concourse.tile as tile
from concourse import bacc, mybir
from concourse import bass_utils

F32 = mybir.dt.float32
BF16 = mybir.dt.bfloat16
NPBF16 = ml_dtypes.bfloat16
AF = mybir.ActivationFunctionType

D = 768
NH = 12
HD = 64
NMLP = 3072
B = 2
T = 4096
EPS = 1e-6
NCORES = 8
GROUPS = [[0, 1, 2, 3], [4, 5, 6, 7]]

TC = T // 4            # tokens per core (1024)
NCH = D // 128         # 6 feature chunks
NPAIR = NH // 2        # 6 head pairs
QKV_CH = 3 * D // 128  # 18
MLP_CH = NMLP // 128   # 24
NTK = T // 128         # 32 key tiles (full sequence)
NTJ = TC // 128        # 8 own-token tiles
HALF = 512             # matmul free-dim tile (one PSUM bank of fp32)
NHALF = TC // HALF     # 2

_CACHE: dict = {}


# --------------------------------------------------------------------------
# device program
# --------------------------------------------------------------------------

def _ln_feature_major(tc, nc, pools, x_tiles, h_tiles_out):
    """LayerNorm over the feature (partition) dim of 6 x [128, TC] fp32 tiles.

    Writes normalized bf16 into h_tiles_out (6 tiles [128, TC]).
    Affine (g, beta) is folded into the following matmul's weights on host.
    """
    sq_pool, norm_pool, stat_psum, small = pools
    ones_f32 = small["ones_f32"]
    ones_bf16 = small["ones_bf16"]

    sq_tiles = []
    for c in range(NCH):
        sq = sq_pool.tile([128, TC], BF16, tag="sq")
        nc.vector.tensor_mul(sq, x_tiles[c], x_tiles[c])
        sq_tiles.append(sq)

    ps_x = [stat_psum.tile([1, HALF], F32, tag="st_x", name="ps_x") for _ in range(NHALF)]
    ps_q = [stat_psum.tile([1, HALF], F32, tag="st_q", name="ps_q") for _ in range(NHALF)]
    for h in range(NHALF):
        sl = slice(h * HALF, (h + 1) * HALF)
        for c in range(NCH):
            nc.tensor.matmul(ps_x[h], ones_f32, x_tiles[c][:, sl],
                             start=(c == 0), stop=(c == NCH - 1))
        for c in range(NCH):
            nc.tensor.matmul(ps_q[h], ones_bf16, sq_tiles[c][:, sl],
                             start=(c == 0), stop=(c == NCH - 1))

    mu = norm_pool.tile([1, TC], F32, tag="mu")
    m2 = norm_pool.tile([1, TC], F32, tag="m2")
    var = norm_pool.tile([1, TC], F32, tag="var")
    lnv = norm_pool.tile([1, TC], F32, tag="lnv")
    rs = norm_pool.tile([1, TC], F32, tag="rs")
    nb = norm_pool.tile([1, TC], F32, tag="nb")
    for h in range(NHALF):
        sl = slice(h * HALF, (h + 1) * HALF)
        nc.vector.tensor_scalar_mul(mu[:, sl], ps_x[h], 1.0 / D)
        nc.vector.tensor_scalar_mul(m2[:, sl], ps_q[h], 1.0 / D)
    nc.vector.tensor_mul(var, mu, mu)
    nc.vector.tensor_sub(var, m2, var)
    # rs = (var + eps) ** -0.5 via Ln/Exp (both live in one ACT table set)
    nc.scalar.activation(lnv, var, AF.Ln, bias=small["eps"])
    nc.scalar.activation(rs, lnv, AF.Exp, scale=-0.5)
    nc.vector.tensor_mul(nb, mu, rs)
    nc.vector.tensor_scalar_mul(nb, nb, -1.0)

    a_bc = norm_pool.tile([128, TC], F32, tag="a_bc")
    b_bc = norm_pool.tile([128, TC], F32, tag="b_bc")
    nc.gpsimd.partition_broadcast(a_bc, rs)
    nc.gpsimd.partition_broadcast(b_bc, nb)

    for c in range(NCH):
        t = norm_pool.tile([128, TC], F32, tag="norm_tmp")
        nc.vector.tensor_mul(t, x_tiles[c], a_bc)
        nc.vector.tensor_add(h_tiles_out[c], t, b_bc)


def _emit(tc, nc, io, loop_n=1):
    if loop_n > 1:
        with tc.For_i(0, loop_n, 1):
            _emit_body(tc, nc, io)
    else:
        _emit_body(tc, nc, io)


def _emit_body(tc, nc, io):
    xT, wqkvT, wprojT, w1T, w2T, bqkv, b1, outT = (
        io["xT"], io["wqkvT"], io["wprojT"], io["w1T"], io["w2T"],
        io["bqkv"], io["b1"], io["outT"])

    xT_v = xT.rearrange("(c p) t -> c p t", p=128)
    out_v = outT.rearrange("(c p) t -> c p t", p=128)

    const = tc.alloc_tile_pool(name="const", bufs=1)
    dram = tc.alloc_tile_pool(name="dram", bufs=1, space="DRAM")

    ident = const.tile([128, 128], F32)
    from concourse.masks import make_identity
    make_identity(nc, ident)
    ones_f32 = const.tile([128, 1], F32)
    ones_bf16 = const.tile([128, 1], BF16)
    nc.any.memset(ones_f32, 1.0)
    nc.any.memset(ones_bf16, 1.0)
    bqkv_sb = const.tile([128, QKV_CH], F32)
    b1_sb = const.tile([128, MLP_CH], F32)
    nc.sync.dma_start(bqkv_sb, bqkv[:])
    nc.sync.dma_start(b1_sb, b1[:])
    wproj_sb = const.tile([128, NCH, D], BF16)
    nc.sync.dma_start(wproj_sb, wprojT.rearrange("(c p) o -> p c o", p=128))
    eps_sb = const.tile([1, 1], F32)
    nc.any.memset(eps_sb, EPS)
    small = {"ones_f32": ones_f32, "ones_bf16": ones_bf16, "eps": eps_sb}

    # DRAM bounce buffers for the K/V all-gather (within 4-core batch group)
    kv_k_in = dram.tile([D, TC], BF16)
    kv_k_out = dram.tile([4 * D, TC], BF16)
    kv_v_in = dram.tile([TC, NH * (HD + 1)], BF16)
    kv_v_out = dram.tile([T, NH * (HD + 1)], BF16)

    # persistent activation pools (alloc order = reverse release order)
    hp = tc.alloc_tile_pool(name="hp", bufs=NCH)
    x2_pool = tc.alloc_tile_pool(name="x2", bufs=NCH)
    xin = tc.alloc_tile_pool(name="xin", bufs=1)
    attno = tc.alloc_tile_pool(name="attno", bufs=NPAIR)
    qt = tc.alloc_tile_pool(name="qt", bufs=NPAIR)

    xin_t = xin.tile([128, NCH, TC], F32, tag="xin")
    nc.sync.dma_start(xin_t, xT.rearrange("(c p) t -> p c t", p=128))
    x_tiles = [xin_t[:, c, :] for c in range(NCH)]

    # ---------------- Phase A: LN1 ----------------
    h_tiles = [hp.tile([128, TC], BF16, tag="h", name="h1t") for _ in range(NCH)]
    with (
        tc.tile_pool(name="sq1", bufs=2) as sq_pool,
        tc.tile_pool(name="norm1", bufs=2) as norm_pool,
        tc.tile_pool(name="stat1", bufs=2, space="PSUM") as stat_psum,
    ):
        _ln_feature_major(tc, nc, (sq_pool, norm_pool, stat_psum, small),
                          x_tiles, h_tiles)

    # ---------------- Phase B: QKV + v transpose + bounce out ----------------
    kv_k_in_v = kv_k_in.rearrange("(c p) t -> c p t", p=128)
    kv_v_in_v = kv_v_in.rearrange("(j p) f -> j p f", p=128)
    with (
        tc.tile_pool(name="wqkv", bufs=1) as wq_pool,
        tc.tile_pool(name="kvloc", bufs=2) as kv_loc,
        tc.tile_pool(name="vtok", bufs=NTJ) as vtok_pool,
        tc.tile_pool(name="mmB", bufs=4, space="PSUM") as mm_psum,
        tc.tile_pool(name="tpB", bufs=2, space="PSUM") as tp_psum,
    ):
        wq_sb = wq_pool.tile([128, NCH, 3 * D], BF16)
        nc.sync.dma_start(wq_sb, wqkvT.rearrange("(c p) o -> p c o", p=128))

        v_tok = []
        for j in range(NTJ):
            vt = vtok_pool.tile([128, NH, HD + 1], BF16, tag="vtok")
            nc.any.memset(vt[:, :, HD:HD + 1], 1.0)
            v_tok.append(vt)

        q_tiles = []
        for oc in range(QKV_CH):
            if oc < NPAIR:
                dst = qt.tile([128, TC], BF16, tag="qt")
                q_tiles.append(dst)
            elif oc < 2 * NPAIR:
                dst = kv_loc.tile([128, TC], BF16, tag="kloc")
            else:
                dst = kv_loc.tile([128, TC], F32, tag="vloc")
            for h in range(NHALF):
                sl = slice(h * HALF, (h + 1) * HALF)
                ps = mm_psum.tile([128, HALF], F32, tag="mm")
                for c in range(NCH):
                    nc.tensor.matmul(ps, wq_sb[:, c, oc * 128:(oc + 1) * 128],
                                     h_tiles[c][:, sl],
                                     start=(c == 0), stop=(c == NCH - 1))
                nc.vector.tensor_scalar_add(dst[:, sl], ps,
                                            bqkv_sb[:, oc:oc + 1])
            if NPAIR <= oc < 2 * NPAIR:
                nc.sync.dma_start(kv_k_in_v[oc - NPAIR], dst)
            elif oc >= 2 * NPAIR:
                c = oc - 2 * NPAIR  # v feature chunk (heads 2c, 2c+1)
                for j in range(NTJ):
                    tp = tp_psum.tile([128, 128], F32, tag="tp")
                    nc.tensor.transpose(tp, dst[:, j * 128:(j + 1) * 128],
                                        ident)
                    nc.vector.tensor_copy(
                        v_tok[j][:, 2 * c:2 * c + 2, 0:HD],
                        tp.rearrange("p (a b) -> p a b", a=2))
        for j in range(NTJ):
            nc.sync.dma_start(kv_v_in_v[j], v_tok[j])

    # ---------------- Phase C: all-gather K/V ----------------
    nc.gpsimd.collective_compute(
        "AllGather", mybir.AluOpType.bypass, replica_groups=GROUPS,
        ins=[kv_k_in[:]], outs=[kv_k_out[:]])
    nc.gpsimd.collective_compute(
        "AllGather", mybir.AluOpType.bypass, replica_groups=GROUPS,
        ins=[kv_v_in[:]], outs=[kv_v_out[:]])

    kv_k_out_v = kv_k_out.rearrange("(g c p) t -> g c p t", c=NCH, p=128)
    kv_v_out_v = kv_v_out.rearrange("(g j p) f -> g j p f", j=NTJ, p=128)

    # ---------------- Phase D: attention ----------------
    with (
        tc.tile_pool(name="vfull", bufs=1) as vfull_pool,
        tc.tile_pool(name="ktp", bufs=2) as ktp_pool,
        tc.tile_pool(name="attn", bufs=3) as attn_pool,
        tc.tile_pool(name="attn_eps", bufs=2) as eps_pool,
        tc.tile_pool(name="qk", bufs=2, space="PSUM") as qk_psum,
        tc.tile_pool(name="av", bufs=4, space="PSUM") as av_psum,
    ):
        vf_t = vfull_pool.tile([128, NTK, NH * (HD + 1)], BF16, tag="vfull")
        vsrc = kv_v_out.rearrange("(g j p) f -> g p j f", j=NTJ, p=128)
        for g in range(4):
            nc.sync.dma_start(vf_t[:, g * NTJ:(g + 1) * NTJ, :], vsrc[g])
        vf_v = vf_t.rearrange("p t (h f) -> p t h f", h=NH)
        v_full = [vf_v[:, t] for t in range(NTK)]

        attno_tiles = []
        for p in range(NPAIR):
            ktp = ktp_pool.tile([128, T], BF16, tag="ktp")
            ksrc = kv_k_out.rearrange("(g c p) t -> p c g t", c=NCH, p=128)
            nc.sync.dma_start(ktp.rearrange("p (g t) -> p g t", g=4),
                              ksrc[:, p])

            avs = [av_psum.tile([HD + 1, HALF], F32, tag="av", name="avt")
                   for _ in range(4)]  # A0 A1 B0 B1
            for ch in range(NTK):
                ksl = slice(ch * 128, (ch + 1) * 128)
                for hh in range(2):
                    head = 2 * p + hh
                    lhs = ktp[hh * 64:(hh + 1) * 64, ksl]
                    ps = qk_psum.tile([128, TC], F32, tag="qk")
                    for h in range(NHALF):
                        nc.tensor.matmul(
                            ps[:, h * HALF:(h + 1) * HALF],
                            lhs,
                            q_tiles[p][hh * 64:(hh + 1) * 64,
                                       h * HALF:(h + 1) * HALF],
                            start=True, stop=True)  # noqa: E501
                    at = attn_pool.tile([128, TC], BF16, tag="attn")
                    nc.scalar.activation(at, ps, AF.Exp)
                    for h in range(NHALF):
                        nc.tensor.matmul(
                            avs[2 * hh + h],
                            v_full[ch][:, head, :],
                            at[:, h * HALF:(h + 1) * HALF],
                            start=(ch == 0), stop=(ch == NTK - 1),
                            skip_group_check=True)

            ao = attno.tile([128, TC], BF16, tag="attno")
            for hh in range(2):
                # drain AV psum to SBUF (DMA cannot read PSUM and DVE cannot
                # shift partitions, so stage through an aligned SBUF copy)
                av_sb = eps_pool.tile([HD + 1, TC], F32, tag="av_sb")
                for h in range(NHALF):
                    sl = slice(h * HALF, (h + 1) * HALF)
                    nc.vector.tensor_copy(av_sb[:, sl], avs[2 * hh + h])
                rd_raw = eps_pool.tile([1, TC], F32, tag="rd_raw")
                rd = eps_pool.tile([1, TC], F32, tag="rden")
                nc.sync.dma_start(rd_raw, av_sb[HD:HD + 1, :])
                nc.vector.reciprocal(rd, rd_raw)
                bc = eps_pool.tile([HD, TC], F32, tag="bc")
                nc.gpsimd.partition_broadcast(bc, rd)
                if hh == 0:
                    nc.vector.tensor_mul(ao[0:HD, :], av_sb[0:HD, :], bc)
                else:
                    tmp = eps_pool.tile([HD, TC], BF16, tag="tmpB")
                    nc.vector.tensor_mul(tmp, av_sb[0:HD, :], bc)
                    nc.sync.dma_start(ao[HD:128, :], tmp)
            attno_tiles.append(ao)

    qt.release()

    # ---------------- Phase E: proj + residual + LN2 ----------------
    x2_tiles = []
    with tc.tile_pool(name="prE", bufs=4, space="PSUM") as pr_psum:
        for oc in range(NCH):
            x2t = x2_pool.tile([128, TC], F32, tag="x2")
            for h in range(NHALF):
                sl = slice(h * HALF, (h + 1) * HALF)
                ps = pr_psum.tile([128, HALF], F32, tag="pr")
                for p in range(NPAIR):
                    nc.tensor.matmul(ps,
                                     wproj_sb[:, p, oc * 128:(oc + 1) * 128],
                                     attno_tiles[p][:, sl],
                                     start=(p == 0), stop=(p == NPAIR - 1))
                nc.vector.tensor_add(x2t[:, sl], x_tiles[oc][:, sl], ps)
            x2_tiles.append(x2t)

    attno.release()
    xin.release()
    w2_pool = tc.alloc_tile_pool(name="w2", bufs=1)
    w2_sb = w2_pool.tile([128, MLP_CH, D], BF16)
    nc.sync.dma_start(w2_sb, w2T.rearrange("(c p) o -> p c o", p=128))



    h2_tiles = [hp.tile([128, TC], BF16, tag="h", name="h2t") for _ in range(NCH)]
    with (
        tc.tile_pool(name="sq2", bufs=2) as sq_pool,
        tc.tile_pool(name="norm2", bufs=2) as norm_pool,
        tc.tile_pool(name="stat2", bufs=2, space="PSUM") as stat_psum,
    ):
        _ln_feature_major(tc, nc, (sq_pool, norm_pool, stat_psum, small),
                          x2_tiles, h2_tiles)

    # ---------------- Phase F: MLP ----------------
    with (
        tc.tile_pool(name="mid", bufs=MLP_CH) as mid_pool,
        tc.tile_pool(name="outp", bufs=2) as out_pool,
        tc.tile_pool(name="mmF", bufs=4, space="PSUM") as mm_psum,
    ):
        with tc.tile_pool(name="w1", bufs=1) as w1_pool:
            w1_sb = w1_pool.tile([128, NCH, NMLP], BF16)
            nc.sync.dma_start(w1_sb, w1T.rearrange("(c p) o -> p c o", p=128))

            mid_tiles = []
            for oc in range(MLP_CH):
                mt = mid_pool.tile([128, TC], BF16, tag="mid")
                for h in range(NHALF):
                    sl = slice(h * HALF, (h + 1) * HALF)
                    ps = mm_psum.tile([128, HALF], F32, tag="mm")
                    for c in range(NCH):
                        nc.tensor.matmul(
                            ps, w1_sb[:, c, oc * 128:(oc + 1) * 128],
                            h2_tiles[c][:, sl],
                            start=(c == 0), stop=(c == NCH - 1))
                    nc.scalar.activation(mt[:, sl], ps, AF.Gelu,
                                         bias=b1_sb[:, oc:oc + 1])
                mid_tiles.append(mt)

        for oc in range(NCH):
            ot = out_pool.tile([128, TC], F32, tag="out")
            for h in range(NHALF):
                sl = slice(h * HALF, (h + 1) * HALF)
                ps = mm_psum.tile([128, HALF], F32, tag="mm")
                for c in range(MLP_CH):
                    nc.tensor.matmul(ps, w2_sb[:, c, oc * 128:(oc + 1) * 128],
                                     mid_tiles[c][:, sl],
                                     start=(c == 0), stop=(c == MLP_CH - 1))
                nc.vector.tensor_add(ot[:, sl], x2_tiles[oc][:, sl], ps)
            nc.sync.dma_start(out_v[oc], ot)

    for pool in (w2_pool, x2_pool, hp, dram, const):
        pool.release()


def _build(loop_n=1):
    nc = bacc.Bacc("TRN2", target_bir_lowering=False, debug=False,
                   num_devices=NCORES)
    io = {
        "xT": nc.dram_tensor("xT", [D, TC], F32, kind="ExternalInput").ap(),
        "wqkvT": nc.dram_tensor("wqkvT", [D, 3 * D], BF16,
                                kind="ExternalInput").ap(),
        "wprojT": nc.dram_tensor("wprojT", [D, D], BF16,
                                 kind="ExternalInput").ap(),
        "w1T": nc.dram_tensor("w1T", [D, NMLP], BF16,
                              kind="ExternalInput").ap(),
        "w2T": nc.dram_tensor("w2T", [NMLP, D], BF16,
                              kind="ExternalInput").ap(),
        "bqkv": nc.dram_tensor("bqkv", [128, QKV_CH], F32,
                               kind="ExternalInput").ap(),
        "b1": nc.dram_tensor("b1", [128, MLP_CH], F32,
                             kind="ExternalInput").ap(),
        "outT": nc.dram_tensor("outT", [D, TC], F32,
                               kind="ExternalOutput").ap(),
    }
    with tile.TileContext(nc) as tc:
        _emit(tc, nc, io, loop_n=loop_n)
    nc.compile()
    return nc


def _get_nc(loop_n=1):
    key = f"nc{loop_n}"
    if key not in _CACHE:
        _CACHE[key] = _build(loop_n=loop_n)
    return _CACHE[key]


def run_timed(inputs, loop_n):
    """Run a variant with the whole kernel wrapped in a hardware loop."""
    import time
    in_maps = _prep_in_maps(**inputs)
    nc = _get_nc(loop_n=loop_n)
    walls = []
    res = None
    for _ in range(6):
        t0 = time.monotonic()
        res = bass_utils.run_bass_kernel_spmd(
            nc, in_maps, core_ids=list(range(NCORES)))
        walls.append(time.monotonic() - t0)
    return _assemble(res.results), walls


# --------------------------------------------------------------------------
# host side
# --------------------------------------------------------------------------

def _prep_in_maps(x, W_qkv, b_qkv, W_proj, b_proj, W1, b1, W2, b2,
                  g1, beta1, g2, beta2):
    f32 = np.float32
    x = np.asarray(x, f32)
    W_qkv = np.asarray(W_qkv, f32)
    b_qkv = np.asarray(b_qkv, f32)
    W_proj = np.asarray(W_proj, f32)
    b_proj = np.asarray(b_proj, f32)
    W1 = np.asarray(W1, f32)
    b1 = np.asarray(b1, f32)
    W2 = np.asarray(W2, f32)
    b2 = np.asarray(b2, f32)
    g1 = np.asarray(g1, f32)
    beta1 = np.asarray(beta1, f32)
    g2 = np.asarray(g2, f32)
    beta2 = np.asarray(beta2, f32)

    assert np.all(b_proj == 0) and np.all(b2 == 0), \
        "nonzero proj/fc2 bias not supported by this kernel build"

    scale = HD ** -0.5
    Wq = W_qkv * g1[None, :]
    bq = b_qkv + W_qkv @ beta1
    Wq[:D] *= scale
    bq = bq.copy()
    bq[:D] *= scale

    W1e = W1 * g2[None, :]
    b1e = b1 + W1 @ beta2

    wqkvT = np.ascontiguousarray(Wq.T).astype(NPBF16)
    wprojT = np.ascontiguousarray(W_proj.T).astype(NPBF16)
    w1T = np.ascontiguousarray(W1e.T).astype(NPBF16)
    w2T = np.ascontiguousarray(W2.T).astype(NPBF16)
    bqkv_dev = np.ascontiguousarray(bq.reshape(QKV_CH, 128).T).astype(f32)
    b1_dev = np.ascontiguousarray(b1e.reshape(MLP_CH, 128).T).astype(f32)

    shared = {"wqkvT": wqkvT, "wprojT": wprojT, "w1T": w1T, "w2T": w2T,
              "bqkv": bqkv_dev, "b1": b1_dev}
    in_maps = []
    for c in range(NCORES):
        b, g = divmod(c, 4)
        xT = np.ascontiguousarray(x[b, g * TC:(g + 1) * TC, :].T)
        in_maps.append({"xT": xT, **shared})
    return in_maps


def _assemble(results):
    out = np.empty((B, T, D), np.float32)
    for c in range(NCORES):
        b, g = divmod(c, 4)
        out[b, g * TC:(g + 1) * TC, :] = results[c]["outT"].T
    return out


def kernel(**inputs) -> np.ndarray:
    in_maps = _prep_in_maps(**inputs)
    nc = _get_nc()
    res = bass_utils.run_bass_kernel_spmd(
        nc, in_maps, core_ids=list(range(NCORES)))
    _CACHE["last_results"] = res
    return _assemble(res.results)


def kernel_sim(**inputs) -> np.ndarray:
    """Run through MultiCoreSim instead of hardware (for testing/timing)."""
    from concourse.bass_interp import MultiCoreSim
    in_maps = _prep_in_maps(**inputs)
    nc = _get_nc()
    sim = MultiCoreSim(nc, num_cores=NCORES, trace=False)
    for c in range(NCORES):
        for name, arr in in_maps[c].items():
            sim.cores[c].tensor(name)[:] = arr
    sim.simulate(check_with_hw=False)
    _CACHE["sim_time_ns"] = sim.global_time
    results = [{"outT": sim.cores[c].mem_tensor("outT")} for c in range(NCORES)]
    return _assemble(results)


# revision 18
# speedup vs baseline: 4609.7205x; 1.0898x over previous
"""Trainium2 Bass kernel for a pre-norm transformer block (attention + MLP).

Problem: x [2, 4096, 768] fp32 through
    x = x + proj(attn(LN1(x)))
    x = x + W2 @ gelu(W1 @ LN2(x))
on 8 NeuronCores.

Sharding: core c handles batch b = c // 4 and sequence slice g = c % 4
(1024 tokens). Each core computes QKV for its own tokens, K/V are
all-gathered within each 4-core batch group (one AllGather through DRAM
bounce buffers), attention/proj/LN2/MLP are fully sequence-parallel.

Layout: activations are kept feature-major ([feature, token], features on
SBUF partitions) so every linear layer is a chain of 128x512 matmuls with
no transposes. LayerNorm statistics (sums over the feature = partition dim)
are computed with ones-vector matmuls on the tensor engine; per-token
scale/shift vectors are broadcast across partitions with GPSIMD
partition_broadcast. Softmax runs on transposed scores ([key, query]) so
that exp'd probabilities feed the AV matmul directly as the moving operand;
the softmax denominator falls out of the AV matmul itself via a ones column
appended to V. LN affine params (g, beta) and the 1/sqrt(hd) query scale
are folded into the weight matrices on the host.
"""

import numpy as np
import ml_dtypes

import concourse.bass as bass
import concourse.tile as tile
from concourse import bacc, mybir
from concourse import bass_utils

F32 = mybir.dt.float32
BF16 = mybir.dt.bfloat16
FP8 = mybir.dt.float8e4
AV_FP8 = True
PSCALE = 32.0  # fp8 softmax numerator scale (cancels in the divide)
NPBF16 = ml_dtypes.bfloat16
AF = mybir.ActivationFunctionType

D = 768
NH = 12
HD = 64
NMLP = 3072
B = 2
T = 4096
EPS = 1e-6
NCORES = 8
GROUPS = [[0, 1, 2, 3], [4, 5, 6, 7]]

TC = T // 4            # tokens per core (1024)
NCH = D // 128         # 6 feature chunks
NPAIR = NH // 2        # 6 head pairs
QKV_CH = 3 * D // 128  # 18
MLP_CH = NMLP // 128   # 24
NTK = T // 128         # 32 key tiles (full sequence)
NTJ = TC // 128        # 8 own-token tiles
HALF = 512             # matmul free-dim tile (one PSUM bank of fp32)
NHALF = TC // HALF     # 2

_CACHE: dict = {}


# --------------------------------------------------------------------------
# device program
# --------------------------------------------------------------------------

def _ln_feature_major(tc, nc, pools, x_tiles, h_tiles_out):
    """LayerNorm over the feature (partition) dim of 6 x [128, TC] fp32 tiles.

    Writes normalized bf16 into h_tiles_out (6 tiles [128, TC]).
    Affine (g, beta) is folded into the following matmul's weights on host.
    """
    sq_pool, norm_pool, stat_psum, small = pools
    ones_f32 = small["ones_f32"]
    ones_bf16 = small["ones_bf16"]

    sq_tiles = []
    for c in range(NCH):
        sq = sq_pool.tile([128, TC], BF16, tag="sq")
        nc.vector.tensor_mul(sq, x_tiles[c], x_tiles[c])
        sq_tiles.append(sq)

    ps_x = [stat_psum.tile([1, HALF], F32, tag="st_x", name="ps_x") for _ in range(NHALF)]
    ps_q = [stat_psum.tile([1, HALF], F32, tag="st_q", name="ps_q") for _ in range(NHALF)]
    for h in range(NHALF):
        sl = slice(h * HALF, (h + 1) * HALF)
        for c in range(NCH):
            nc.tensor.matmul(ps_x[h], ones_f32, x_tiles[c][:, sl],
                             start=(c == 0), stop=(c == NCH - 1))
        for c in range(NCH):
            nc.tensor.matmul(ps_q[h], ones_bf16, sq_tiles[c][:, sl],
                             start=(c == 0), stop=(c == NCH - 1))

    mu = norm_pool.tile([1, TC], F32, tag="mu")
    m2 = norm_pool.tile([1, TC], F32, tag="m2")
    var = norm_pool.tile([1, TC], F32, tag="var")
    lnv = norm_pool.tile([1, TC], F32, tag="lnv")
    rs = norm_pool.tile([1, TC], F32, tag="rs")
    nb = norm_pool.tile([1, TC], F32, tag="nb")
    for h in range(NHALF):
        sl = slice(h * HALF, (h + 1) * HALF)
        nc.vector.tensor_scalar_mul(mu[:, sl], ps_x[h], 1.0 / D)
        nc.vector.tensor_scalar_mul(m2[:, sl], ps_q[h], 1.0 / D)
    nc.vector.tensor_mul(var, mu, mu)
    nc.vector.tensor_sub(var, m2, var)
    # rs = (var + eps) ** -0.5 via Ln/Exp (both live in one ACT table set)
    nc.scalar.activation(lnv, var, AF.Ln, bias=small["eps"])
    nc.scalar.activation(rs, lnv, AF.Exp, scale=-0.5)
    nc.vector.tensor_mul(nb, mu, rs)
    nc.vector.tensor_scalar_mul(nb, nb, -1.0)

    a_bc = norm_pool.tile([128, TC], F32, tag="a_bc")
    b_bc = norm_pool.tile([128, TC], F32, tag="b_bc")
    nc.gpsimd.partition_broadcast(a_bc, rs)
    nc.gpsimd.partition_broadcast(b_bc, nb)

    for c in range(NCH):
        t = norm_pool.tile([128, TC], F32, tag="norm_tmp")
        nc.vector.tensor_mul(t, x_tiles[c], a_bc)
        nc.vector.tensor_add(h_tiles_out[c], t, b_bc)


def _emit(tc, nc, io, loop_n=1):
    if loop_n > 1:
        with tc.For_i(0, loop_n, 1):
            _emit_body(tc, nc, io)
    else:
        _emit_body(tc, nc, io)


def _emit_body(tc, nc, io):
    xT, wqkvT, wprojT, w1T, w2T, bqkv, b1, outT = (
        io["xT"], io["wqkvT"], io["wprojT"], io["w1T"], io["w2T"],
        io["bqkv"], io["b1"], io["outT"])

    xT_v = xT.rearrange("(c p) t -> c p t", p=128)
    out_v = outT.rearrange("(c p) t -> c p t", p=128)

    const = tc.alloc_tile_pool(name="const", bufs=1)
    dram = tc.alloc_tile_pool(name="dram", bufs=1, space="DRAM")

    ident = const.tile([128, 128], F32)
    from concourse.masks import make_identity
    make_identity(nc, ident)
    ones_f32 = const.tile([128, 1], F32)
    ones_bf16 = const.tile([128, 1], BF16)
    nc.any.memset(ones_f32, 1.0)
    nc.any.memset(ones_bf16, 1.0)
    bqkv_sb = const.tile([128, QKV_CH], F32)
    b1_sb = const.tile([128, MLP_CH], F32)
    nc.sync.dma_start(bqkv_sb, bqkv[:])
    nc.sync.dma_start(b1_sb, b1[:])
    wproj_sb = const.tile([128, NCH, D], BF16)
    nc.sync.dma_start(wproj_sb, wprojT.rearrange("(c p) o -> p c o", p=128))
    eps_sb = const.tile([1, 1], F32)
    nc.any.memset(eps_sb, EPS)
    lnsc_sb = const.tile([128, 1], F32)
    import math
    nc.any.memset(lnsc_sb, math.log(PSCALE))
    small = {"ones_f32": ones_f32, "ones_bf16": ones_bf16, "eps": eps_sb}

    # DRAM bounce buffers for the K/V all-gather (within 4-core batch group)
    kv_k_in = dram.tile([D, TC], BF16)
    kv_k_out = dram.tile([4 * D, TC], BF16)
    kv_v_in = dram.tile([TC, NH * (HD + 1)], FP8 if AV_FP8 else BF16)
    kv_v_out = dram.tile([T, NH * (HD + 1)], FP8 if AV_FP8 else BF16)

    # persistent activation pools (alloc order = reverse release order)
    hp = tc.alloc_tile_pool(name="hp", bufs=NCH)
    x2_pool = tc.alloc_tile_pool(name="x2", bufs=NCH)
    xin = tc.alloc_tile_pool(name="xin", bufs=1)
    attno = tc.alloc_tile_pool(name="attno", bufs=NPAIR)
    qt = tc.alloc_tile_pool(name="qt", bufs=NPAIR)

    xin_t = xin.tile([128, NCH, TC], F32, tag="xin")
    nc.sync.dma_start(xin_t, xT.rearrange("(c p) t -> p c t", p=128))
    x_tiles = [xin_t[:, c, :] for c in range(NCH)]

    # ---------------- Phase A: LN1 ----------------
    h_tiles = [hp.tile([128, TC], BF16, tag="h", name="h1t") for _ in range(NCH)]
    with (
        tc.tile_pool(name="sq1", bufs=2) as sq_pool,
        tc.tile_pool(name="norm1", bufs=2) as norm_pool,
        tc.tile_pool(name="stat1", bufs=2, space="PSUM") as stat_psum,
    ):
        _ln_feature_major(tc, nc, (sq_pool, norm_pool, stat_psum, small),
                          x_tiles, h_tiles)

    # ---------------- Phase B: QKV + v transpose + bounce out ----------------
    kv_k_in_v = kv_k_in.rearrange("(c p) t -> c p t", p=128)
    kv_v_in_v = kv_v_in.rearrange("(j p) f -> j p f", p=128)
    with (
        tc.tile_pool(name="wqkv", bufs=1) as wq_pool,
        tc.tile_pool(name="kvloc", bufs=2) as kv_loc,
        tc.tile_pool(name="vtok", bufs=NTJ) as vtok_pool,
        tc.tile_pool(name="mmB", bufs=4, space="PSUM") as mm_psum,
        tc.tile_pool(name="tpB", bufs=2, space="PSUM") as tp_psum,
    ):
        wq_sb = wq_pool.tile([128, NCH, 3 * D], BF16)
        nc.sync.dma_start(wq_sb, wqkvT.rearrange("(c p) o -> p c o", p=128))

        v_tok = []
        for j in range(NTJ):
            vt = vtok_pool.tile([128, NH, HD + 1], FP8 if AV_FP8 else BF16,
                                tag="vtok")
            nc.any.memset(vt[:, :, HD:HD + 1], 1.0)
            v_tok.append(vt)

        q_tiles = []
        for oc in range(QKV_CH):
            if oc < NPAIR:
                dst = qt.tile([128, TC], BF16, tag="qt")
                q_tiles.append(dst)
            elif oc < 2 * NPAIR:
                dst = kv_loc.tile([128, TC], BF16, tag="kloc")
            else:
                dst = kv_loc.tile([128, TC], F32, tag="vloc")
            for h in range(NHALF):
                sl = slice(h * HALF, (h + 1) * HALF)
                ps = mm_psum.tile([128, HALF], F32, tag="mm")
                for c in range(NCH):
                    nc.tensor.matmul(ps, wq_sb[:, c, oc * 128:(oc + 1) * 128],
                                     h_tiles[c][:, sl],
                                     start=(c == 0), stop=(c == NCH - 1))
                nc.vector.tensor_scalar_add(dst[:, sl], ps,
                                            bqkv_sb[:, oc:oc + 1])
            if NPAIR <= oc < 2 * NPAIR:
                nc.sync.dma_start(kv_k_in_v[oc - NPAIR], dst)
            elif oc >= 2 * NPAIR:
                c = oc - 2 * NPAIR  # v feature chunk (heads 2c, 2c+1)
                for j in range(NTJ):
                    tp = tp_psum.tile([128, 128], F32, tag="tp")
                    nc.tensor.transpose(tp, dst[:, j * 128:(j + 1) * 128],
                                        ident)
                    nc.vector.tensor_copy(
                        v_tok[j][:, 2 * c:2 * c + 2, 0:HD],
                        tp.rearrange("p (a b) -> p a b", a=2))
        for j in range(NTJ):
            nc.sync.dma_start(kv_v_in_v[j], v_tok[j])

    # ---------------- Phase C: all-gather K/V ----------------
    nc.gpsimd.collective_compute(
        "AllGather", mybir.AluOpType.bypass, replica_groups=GROUPS,
        ins=[kv_k_in[:]], outs=[kv_k_out[:]])
    nc.gpsimd.collective_compute(
        "AllGather", mybir.AluOpType.bypass, replica_groups=GROUPS,
        ins=[kv_v_in[:]], outs=[kv_v_out[:]])

    kv_k_out_v = kv_k_out.rearrange("(g c p) t -> g c p t", c=NCH, p=128)
    kv_v_out_v = kv_v_out.rearrange("(g j p) f -> g j p f", j=NTJ, p=128)

    # ---------------- Phase D: attention ----------------
    with (
        tc.tile_pool(name="vfull", bufs=1) as vfull_pool,
        tc.tile_pool(name="ktp", bufs=2) as ktp_pool,
        tc.tile_pool(name="attn", bufs=3) as attn_pool,
        tc.tile_pool(name="attn_eps", bufs=2) as eps_pool,
        tc.tile_pool(name="qk", bufs=2, space="PSUM") as qk_psum,
        tc.tile_pool(name="av", bufs=4, space="PSUM") as av_psum,
    ):
        vf_t = vfull_pool.tile([128, NTK, NH * (HD + 1)],
                               FP8 if AV_FP8 else BF16, tag="vfull")
        vsrc = kv_v_out.rearrange("(g j p) f -> g p j f", j=NTJ, p=128)
        for g in range(4):
            nc.sync.dma_start(vf_t[:, g * NTJ:(g + 1) * NTJ, :], vsrc[g])
        vf_v = vf_t.rearrange("p t (h f) -> p t h f", h=NH)
        v_full = [vf_v[:, t] for t in range(NTK)]

        attno_tiles = []
        for p in range(NPAIR):
            ktp = ktp_pool.tile([128, T], BF16, tag="ktp")
            ksrc = kv_k_out.rearrange("(g c p) t -> p c g t", c=NCH, p=128)
            nc.sync.dma_start(ktp.rearrange("p (g t) -> p g t", g=4),
                              ksrc[:, p])

            avs = [av_psum.tile([HD + 1, HALF], F32, tag="av", name="avt")
                   for _ in range(4)]  # A0 A1 B0 B1
            if not AV_FP8:
                for ch in range(NTK):
                    ksl = slice(ch * 128, (ch + 1) * 128)
                    for hh in range(2):
                        head = 2 * p + hh
                        lhs = ktp[hh * 64:(hh + 1) * 64, ksl]
                        ps = qk_psum.tile([128, TC], F32, tag="qk")
                        for h in range(NHALF):
                            nc.tensor.matmul(
                                ps[:, h * HALF:(h + 1) * HALF],
                                lhs,
                                q_tiles[p][hh * 64:(hh + 1) * 64,
                                           h * HALF:(h + 1) * HALF],
                                start=True, stop=True)  # noqa: E501
                        at = attn_pool.tile([128, TC], BF16, tag="attn")
                        nc.scalar.activation(at, ps, AF.Exp)
                        for h in range(NHALF):
                            nc.tensor.matmul(
                                avs[2 * hh + h],
                                v_full[ch][:, head, :],
                                at[:, h * HALF:(h + 1) * HALF],
                                start=(ch == 0), stop=(ch == NTK - 1),
                                skip_group_check=True)
            else:
                for chp in range(NTK // 2):
                    for hh in range(2):
                        head = 2 * p + hh
                        at = attn_pool.tile([128, 2, TC], FP8, tag="attn")
                        for j in range(2):
                            ch = 2 * chp + j
                            ksl = slice(ch * 128, (ch + 1) * 128)
                            lhs = ktp[hh * 64:(hh + 1) * 64, ksl]
                            ps = qk_psum.tile([128, TC], F32, tag="qk")
                            for h in range(NHALF):
                                nc.tensor.matmul(
                                    ps[:, h * HALF:(h + 1) * HALF],
                                    lhs,
                                    q_tiles[p][hh * 64:(hh + 1) * 64,
                                               h * HALF:(h + 1) * HALF],
                                    start=True, stop=True)  # noqa: E501
                            # probs * PSCALE in fp8e4m3
                            nc.scalar.activation(at[:, j, :], ps, AF.Exp,
                                                 bias=lnsc_sb[:, 0:1])
                        for h in range(NHALF):
                            nc.tensor.matmul(
                                avs[2 * hh + h],
                                vf_v[:, 2 * chp:2 * chp + 2, head, :],
                                at[:, :, h * HALF:(h + 1) * HALF],
                                start=(chp == 0), stop=(chp == NTK // 2 - 1),
                                skip_group_check=True,
                                perf_mode=mybir.MatmulPerfMode.DoubleRow)

            ao = attno.tile([128, TC], BF16, tag="attno")
            for hh in range(2):
                # drain AV psum to SBUF (DMA cannot read PSUM and DVE cannot
                # shift partitions, so stage through an aligned SBUF copy)
                av_sb = eps_pool.tile([HD + 1, TC], F32, tag="av_sb")
                for h in range(NHALF):
                    sl = slice(h * HALF, (h + 1) * HALF)
                    nc.vector.tensor_copy(av_sb[:, sl], avs[2 * hh + h])
                rd_raw = eps_pool.tile([1, TC], F32, tag="rd_raw")
                rd = eps_pool.tile([1, TC], F32, tag="rden")
                nc.sync.dma_start(rd_raw, av_sb[HD:HD + 1, :])
                nc.vector.reciprocal(rd, rd_raw)
                bc = eps_pool.tile([HD, TC], F32, tag="bc")
                nc.gpsimd.partition_broadcast(bc, rd)
                if hh == 0:
                    nc.vector.tensor_mul(ao[0:HD, :], av_sb[0:HD, :], bc)
                else:
                    tmp = eps_pool.tile([HD, TC], BF16, tag="tmpB")
                    nc.vector.tensor_mul(tmp, av_sb[0:HD, :], bc)
                    nc.sync.dma_start(ao[HD:128, :], tmp)
            attno_tiles.append(ao)

    qt.release()

    # ---------------- Phase E: proj + residual + LN2 ----------------
    x2_tiles = []
    with tc.tile_pool(name="prE", bufs=4, space="PSUM") as pr_psum:
        for oc in range(NCH):
            x2t = x2_pool.tile([128, TC], F32, tag="x2")
            for h in range(NHALF):
                sl = slice(h * HALF, (h + 1) * HALF)
                ps = pr_psum.tile([128, HALF], F32, tag="pr")
                for p in range(NPAIR):
                    nc.tensor.matmul(ps,
                                     wproj_sb[:, p, oc * 128:(oc + 1) * 128],
                                     attno_tiles[p][:, sl],
                                     start=(p == 0), stop=(p == NPAIR - 1))
                nc.vector.tensor_add(x2t[:, sl], x_tiles[oc][:, sl], ps)
            x2_tiles.append(x2t)

    attno.release()
    xin.release()
    w2_pool = tc.alloc_tile_pool(name="w2", bufs=1)
    w2_sb = w2_pool.tile([128, MLP_CH, D], BF16)
    nc.sync.dma_start(w2_sb, w2T.rearrange("(c p) o -> p c o", p=128))



    h2_tiles = [hp.tile([128, TC], BF16, tag="h", name="h2t") for _ in range(NCH)]
    with (
        tc.tile_pool(name="sq2", bufs=2) as sq_pool,
        tc.tile_pool(name="norm2", bufs=2) as norm_pool,
        tc.tile_pool(name="stat2", bufs=2, space="PSUM") as stat_psum,
    ):
        _ln_feature_major(tc, nc, (sq_pool, norm_pool, stat_psum, small),
                          x2_tiles, h2_tiles)

    # ---------------- Phase F: MLP ----------------
    with (
        tc.tile_pool(name="mid", bufs=MLP_CH) as mid_pool,
        tc.tile_pool(name="outp", bufs=2) as out_pool,
        tc.tile_pool(name="mmF", bufs=4, space="PSUM") as mm_psum,
    ):
        with tc.tile_pool(name="w1", bufs=1) as w1_pool:
            w1_sb = w1_pool.tile([128, NCH, NMLP], BF16)
            nc.sync.dma_start(w1_sb, w1T.rearrange("(c p) o -> p c o", p=128))

            mid_tiles = []
            for oc in range(MLP_CH):
                mt = mid_pool.tile([128, TC], BF16, tag="mid")
                for h in range(NHALF):
                    sl = slice(h * HALF, (h + 1) * HALF)
                    ps = mm_psum.tile([128, HALF], F32, tag="mm")
                    for c in range(NCH):
                        nc.tensor.matmul(
                            ps, w1_sb[:, c, oc * 128:(oc + 1) * 128],
                            h2_tiles[c][:, sl],
                            start=(c == 0), stop=(c == NCH - 1))
                    nc.scalar.activation(mt[:, sl], ps, AF.Gelu,
                                         bias=b1_sb[:, oc:oc + 1])
                mid_tiles.append(mt)

        for oc in range(NCH):
            ot = out_pool.tile([128, TC], F32, tag="out")
            for h in range(NHALF):
                sl = slice(h * HALF, (h + 1) * HALF)
                ps = mm_psum.tile([128, HALF], F32, tag="mm")
                for c in range(MLP_CH):
                    nc.tensor.matmul(ps, w2_sb[:, c, oc * 128:(oc + 1) * 128],
                                     mid_tiles[c][:, sl],
                                     start=(c == 0), stop=(c == MLP_CH - 1))
                nc.vector.tensor_add(ot[:, sl], x2_tiles[oc][:, sl], ps)
            nc.sync.dma_start(out_v[oc], ot)

    for pool in (w2_pool, x2_pool, hp, dram, const):
        pool.release()


def _build(loop_n=1):
    nc = bacc.Bacc("TRN2", target_bir_lowering=False, debug=False,
                   num_devices=NCORES)
    io = {
        "xT": nc.dram_tensor("xT", [D, TC], F32, kind="ExternalInput").ap(),
        "wqkvT": nc.dram_tensor("wqkvT", [D, 3 * D], BF16,
                                kind="ExternalInput").ap(),
        "wprojT": nc.dram_tensor("wprojT", [D, D], BF16,
                                 kind="ExternalInput").ap(),
        "w1T": nc.dram_tensor("w1T", [D, NMLP], BF16,
                              kind="ExternalInput").ap(),
        "w2T": nc.dram_tensor("w2T", [NMLP, D], BF16,
                              kind="ExternalInput").ap(),
        "bqkv": nc.dram_tensor("bqkv", [128, QKV_CH], F32,
                               kind="ExternalInput").ap(),
        "b1": nc.dram_tensor("b1", [128, MLP_CH], F32,
                             kind="ExternalInput").ap(),
        "outT": nc.dram_tensor("outT", [D, TC], F32,
                               kind="ExternalOutput").ap(),
    }
    with tile.TileContext(nc) as tc:
        _emit(tc, nc, io, loop_n=loop_n)
    nc.compile()
    return nc


def _get_nc(loop_n=1):
    key = f"nc{loop_n}"
    if key not in _CACHE:
        _CACHE[key] = _build(loop_n=loop_n)
    return _CACHE[key]


def run_timed(inputs, loop_n):
    """Run a variant with the whole kernel wrapped in a hardware loop."""
    import time
    in_maps = _prep_in_maps(**inputs)
    nc = _get_nc(loop_n=loop_n)
    walls = []
    res = None
    for _ in range(6):
        t0 = time.monotonic()
        res = bass_utils.run_bass_kernel_spmd(
            nc, in_maps, core_ids=list(range(NCORES)))
        walls.append(time.monotonic() - t0)
    return _assemble(res.results), walls


# --------------------------------------------------------------------------
# host side
# --------------------------------------------------------------------------

def _prep_in_maps(x, W_qkv, b_qkv, W_proj, b_proj, W1, b1, W2, b2,
                  g1, beta1, g2, beta2):
    f32 = np.float32
    x = np.asarray(x, f32)
    W_qkv = np.asarray(W_qkv, f32)
    b_qkv = np.asarray(b_qkv, f32)
    W_proj = np.asarray(W_proj, f32)
    b_proj = np.asarray(b_proj, f32)
    W1 = np.asarray(W1, f32)
    b1 = np.asarray(b1, f32)
    W2 = np.asarray(W2, f32)
    b2 = np.asarray(b2, f32)
    g1 = np.asarray(g1, f32)
    beta1 = np.asarray(beta1, f32)
    g2 = np.asarray(g2, f32)
    beta2 = np.asarray(beta2, f32)

    assert np.all(b_proj == 0) and np.all(b2 == 0), \
        "nonzero proj/fc2 bias not supported by this kernel build"

    scale = HD ** -0.5
    Wq = W_qkv * g1[None, :]
    bq = b_qkv + W_qkv @ beta1
    Wq[:D] *= scale
    bq = bq.copy()
    bq[:D] *= scale

    W1e = W1 * g2[None, :]
    b1e = b1 + W1 @ beta2

    wqkvT = np.ascontiguousarray(Wq.T).astype(NPBF16)
    wprojT = np.ascontiguousarray(W_proj.T).astype(NPBF16)
    w1T = np.ascontiguousarray(W1e.T).astype(NPBF16)
    w2T = np.ascontiguousarray(W2.T).astype(NPBF16)
    bqkv_dev = np.ascontiguousarray(bq.reshape(QKV_CH, 128).T).astype(f32)
    b1_dev = np.ascontiguousarray(b1e.reshape(MLP_CH, 128).T).astype(f32)

    shared = {"wqkvT": wqkvT, "wprojT": wprojT, "w1T": w1T, "w2T": w2T,
              "bqkv": bqkv_dev, "b1": b1_dev}
    in_maps = []
    for c in range(NCORES):
        b, g = divmod(c, 4)
        xT = np.ascontiguousarray(x[b, g * TC:(g + 1) * TC, :].T)
        in_maps.append({"xT": xT, **shared})
    return in_maps


def _assemble(results):
    out = np.empty((B, T, D), np.float32)
    for c in range(NCORES):
        b, g = divmod(c, 4)
        out[b, g * TC:(g + 1) * TC, :] = results[c]["outT"].T
    return out


def kernel(**inputs) -> np.ndarray:
    in_maps = _prep_in_maps(**inputs)
    nc = _get_nc()
    res = bass_utils.run_bass_kernel_spmd(
        nc, in_maps, core_ids=list(range(NCORES)))
    _CACHE["last_results"] = res
    return _assemble(res.results)


def kernel_sim(**inputs) -> np.ndarray:
    """Run through MultiCoreSim instead of hardware (for testing/timing)."""
    from concourse.bass_interp import MultiCoreSim
    in_maps = _prep_in_maps(**inputs)
    nc = _get_nc()
    sim = MultiCoreSim(nc, num_cores=NCORES, trace=False)
    for c in range(NCORES):
        for name, arr in in_maps[c].items():
            sim.cores[c].tensor(name)[:] = arr
    sim.simulate(check_with_hw=False)
    _CACHE["sim_time_ns"] = sim.global_time
    results = [{"outT": sim.cores[c].mem_tensor("outT")} for c in range(NCORES)]
    return _assemble(results)
